# revision 1
# baseline (speedup 1.0000x reference)
"""LongFormer sliding-window attention on 8 Trainium2 NeuronCores.

Sharding: batch*heads data-parallel. 24 (batch, head) pairs -> 8 cores,
each core owns one batch (core//4) and 3 consecutive heads (3*(core%4)).
No collectives: each core computes Q/K/V projections for its heads over
the full sequence, then banded attention, then writes its [S, 192] slice
of the output.

Per-core kernel layout strategy:
  - x [4096, 768] is transposed on-chip (PE transpose) to xT [768, 4096].
  - Q,K projected directly into transposed layout qT/kT [64, 4096] per
    head by using the weight matrix as the stationary operand (heads are
    packed in pairs to fill 128 output partitions).
  - scores are computed TRANSPOSED: scoresT[k, qi] = kT_block.T-free
    matmul with lhsT=kT block [64,128], rhs=qT chunk [64,256].  Softmax
    along k (the partition-tiled dim) then needs no transposes anywhere:
    exp() is elementwise, the denominator comes from appending a
    ones-column to V (so PV's output column 64 is sum_k E[k, qi]), and
    the PV matmul out[qi, d] = sum_k E[k,qi]*v[k,d] takes E tiles
    directly as the stationary operand.
  - The band mask is handled structurally: only the 5 (of 6) valid
    128-key tiles per 256-query chunk are computed/accumulated, and the
    4 triangular diagonal blocks are masked multiplicatively on E with
    two constant [128,128] triangle masks.
  - matmuls run as float32r (full-rate fp32 streaming); E and V are
    fp16 for the PV stage (1 cycle/row at N=65, 4x DVE mask mode).
"""

import os
import sys

import numpy as np

sys.path.insert(0, "/opt/trn_rl_repo")

import concourse.bass as bass  # noqa: E402
import concourse.tile as tile  # noqa: E402
from concourse import bacc, mybir  # noqa: E402
from concourse import bass_utils  # noqa: E402

B, S, E = 2, 4096, 768
H, D = 12, 64
W2 = 256            # one-sided window w
C = S // W2         # 16 chunks of 256 queries
HPC = 3             # heads per core
N_CORES = 8

f32 = mybir.dt.float32
f32r = mybir.dt.float32r
f16 = mybir.dt.float16

KT = 6              # 768 = 6 k-tiles of 128
NT = 8              # 4096 = 8 n-tiles of 512
RT = 32             # 4096 = 32 row-tiles of 128
VW = 65 * HPC       # packed v width: 3 heads x (64 dims + ones col)


def _build_body(tc, aps):
    nc = tc.nc
    xt_d, wqk_d, bqk_d, wv_d, wvr_d, masks_d, ones_d, out_d = aps

    from contextlib import ExitStack
    ctx = ExitStack()
    sb = ctx.enter_context(tc.tile_pool(name="sb", bufs=1))
    xnat_p = ctx.enter_context(tc.tile_pool(name="xnat", bufs=3))
    e_p = ctx.enter_context(tc.tile_pool(name="ep", bufs=10))
    out_p = ctx.enter_context(tc.tile_pool(name="outp", bufs=4))
    ps = ctx.enter_context(tc.tile_pool(name="ps", bufs=6, space="PSUM"))
    ps_o = ctx.enter_context(tc.tile_pool(name="pso", bufs=2, space="PSUM"))

    # ---- persistent SBUF tensors (one big tile each, column-sliced) ----
    mask_l = sb.tile([128, 128], f16, tag="mask_l")
    nc.sync.dma_start(mask_l[:], masks_d[0])
    mask_u = sb.tile([128, 128], f16, tag="mask_u")
    nc.sync.dma_start(mask_u[:], masks_d[1])
    wqk = sb.tile([128, KT * 384], f32r, tag="wqk")
    for kt in range(KT):
        nc.sync.dma_start(wqk[:, kt * 384:(kt + 1) * 384],
                          wqk_d[kt * 128:(kt + 1) * 128, :])
    wv = sb.tile([128, KT * 260], f32r, tag="wv")
    for kt in range(KT):
        nc.sync.dma_start(wv[:, kt * 260:(kt + 1) * 260],
                          wv_d[kt * 128:(kt + 1) * 128, :])
    wvr = sb.tile([1, 260], f32r, tag="wvr")
    nc.sync.dma_start(wvr[:], wvr_d[:])
    bqk = sb.tile([128, 4], f32, tag="bqk")
    for g in range(4):
        nc.sync.dma_start(bqk[:, g:g + 1],
                          bqk_d[g].rearrange("(p o) -> p o", o=1))
    ones1 = sb.tile([1, 128], f32r, tag="ones1")
    nc.sync.dma_start(ones1[:], ones_d[:])

    qkT = sb.tile([128, 4 * S], f32r, tag="qkT")         # 64 KiB/part
    vsb = sb.tile([128, RT * VW], f16, tag="vsb")       # 12.2 KiB/part

    def qkT_s(g, lo, n, p0=0, pn=128):
        return qkT[p0:p0 + pn, g * S + lo: g * S + lo + n]

    # ---- phase 1+2: transpose x and project, one 512-token stripe at a
    # time (xT slice is transient).  Projection groups: g0 = Wq heads01
    # (M=128), g1 = Wk heads01 (M=128), g2 = Wq h2 (M=64), g3 = Wk h2
    # (M=64) -- head2 q/k kept at base partition 0 so QK matmuls match.
    for nt in range(NT):
        xTn = xnat_p.tile([128, KT * 512], f32r, tag="xTn", bufs=2)
        for kt in range(KT):
            nc.sync.dma_start(
                xTn[:, kt * 512:(kt + 1) * 512],
                xt_d[kt * 128:(kt + 1) * 128, nt * 512:(nt + 1) * 512])
        for g in range(4):
            gm = 128 if g < 2 else 64
            gc0 = g * 128 if g < 2 else 256 + (g - 2) * 64
            pq = ps.tile([128, 512], f32, tag="ps")
            for kt in range(KT):
                nc.tensor.matmul(
                    pq[0:gm, :],
                    wqk[:, kt * 384 + gc0: kt * 384 + gc0 + gm],
                    xTn[:, kt * 512:(kt + 1) * 512],
                    start=(kt == 0), stop=(kt == KT - 1),
                )
            nc.vector.tensor_scalar_add(
                qkT_s(g, nt * 512, 512, pn=gm), pq[0:gm, :], bqk[0:gm, g:g + 1])
        # V projection for this stripe's 4 row tiles
        for rt4 in range(4):
            rt = nt * 4 + rt4
            pv = ps.tile([128, 512], f32, tag="ps")
            for kt in range(KT):
                nc.tensor.matmul(
                    pv[:, 0:260],
                    xTn[:, kt * 512 + rt4 * 128: kt * 512 + rt4 * 128 + 128],
                    wv[:, kt * 260:(kt + 1) * 260],
                    start=(kt == 0), stop=False,
                )
            nc.tensor.matmul(
                pv[:, 0:260], ones1[:], wvr[:],
                start=False, stop=True,
            )
            nc.vector.tensor_copy(vsb[:, rt * VW: rt * VW + VW], pv[:, 0:VW])

    # ---- phase 3: banded attention ----
    # head h slices: h in {0,1}: qT = g0 rows 64h..64h+64, kT = g1 same
    # rows; h=2: qT = g2 rows 0:64, kT = g3 rows 0:64.
    def q_slice(h, lo, n):
        if h < 2:
            return qkT_s(0, lo, n, p0=64 * h, pn=64)
        return qkT_s(2, lo, n, p0=0, pn=64)

    def k_slice(h, lo, n):
        if h < 2:
            return qkT_s(1, lo, n, p0=64 * h, pn=64)
        return qkT_s(3, lo, n, p0=0, pn=64)

    for c in range(C):
        ots = [out_p.tile([128, 192], f32, tag="ot", name="ot") for _ in range(2)]
        for hi in range(HPC):
            # valid relative key tiles t (of 6): absolute tile 2(c-1)+t
            tmin = 2 if c == 0 else 0
            tmax = 3 if c == C - 1 else 5
            etile = {}
            for t in range(tmin, tmax + 1):
                kt_abs = 2 * (c - 1) + t
                # query column span covered by this key tile
                qlo, qn = (0, 128) if t == 0 else ((128, 128) if t == 5 else (0, 256))
                pt = ps.tile([128, 512], f32, tag="ps")
                nc.tensor.matmul(
                    pt[:, 0:qn],
                    k_slice(hi, kt_abs * 128, 128),
                    q_slice(hi, c * 256 + qlo, qn),
                    start=True, stop=True,
                )
                et = e_p.tile([128, 256], f16, tag="et")
                nc.scalar.activation(
                    et[:, 0:qn], pt[:, 0:qn],
                    mybir.ActivationFunctionType.Exp, scale=0.125)
                etile[t] = (et, qn)
                # triangle masks on the diagonal blocks
                if t == 0:
                    nc.vector.tensor_mul(et[:, 0:128], et[:, 0:128], mask_l[:])
                elif t == 1:
                    nc.vector.tensor_mul(et[:, 128:256], et[:, 128:256], mask_l[:])
                elif t == 4:
                    nc.vector.tensor_mul(et[:, 0:128], et[:, 0:128], mask_u[:])
                elif t == 5:
                    nc.vector.tensor_mul(et[:, 0:128], et[:, 0:128], mask_u[:])
            for qh in range(2):
                ts = [t for t in range(tmin, tmax + 1)
                      if (t <= 4 if qh == 0 else t >= 1)]
                po = ps_o.tile([128, 65], f32, tag="po")
                for i, t in enumerate(ts):
                    et, qn = etile[t]
                    if qh == 0 or t == 5:
                        esl = et[:, 0:128]
                    else:
                        esl = et[:, 128:256]
                    kt_abs = 2 * (c - 1) + t
                    nc.tensor.matmul(
                        po[:],
                        esl,
                        vsb[:, kt_abs * VW + hi * 65: kt_abs * VW + (hi + 1) * 65],
                        start=(i == 0), stop=(i == len(ts) - 1),
                    )
                rec = e_p.tile([128, 1], f32, tag="rec")
                nc.vector.reciprocal(rec[:], po[:, 64:65])
                nc.vector.tensor_scalar_mul(
                    ots[qh][:, hi * 64:(hi + 1) * 64], po[:, 0:64], rec[:])
        for qh in range(2):
            nc.sync.dma_start(
                out_d[c * 256 + qh * 128: c * 256 + qh * 128 + 128, :],
                ots[qh][:])
    ctx.close()


def build_program():
    nc = bacc.Bacc("TRN2", target_bir_lowering=False, debug=False)
    xt_d = nc.dram_tensor("xt", [E, S], f32r, kind="ExternalInput").ap()
    wqk_d = nc.dram_tensor("wqk", [E, 384], f32r, kind="ExternalInput").ap()
    bqk_d = nc.dram_tensor("bqk", [4, 128], f32, kind="ExternalInput").ap()
    wv_d = nc.dram_tensor("wv", [E, 260], f32r, kind="ExternalInput").ap()
    wvr_d = nc.dram_tensor("wvr", [1, 260], f32r, kind="ExternalInput").ap()
    masks_d = nc.dram_tensor("masks", [2, 128, 128], f16, kind="ExternalInput").ap()
    ones_d = nc.dram_tensor("onesr", [1, 128], f32r, kind="ExternalInput").ap()
    out_d = nc.dram_tensor("out", [S, 192], f32, kind="ExternalOutput").ap()
    with tile.TileContext(nc) as tc:
        _build_body(tc, (xt_d, wqk_d, bqk_d, wv_d, wvr_d, masks_d, ones_d, out_d))
    nc.compile()
    return nc


def make_in_maps(hidden_states, Wq, bq, Wk, bk, Wv, bv):
    hs = np.asarray(hidden_states, np.float32)
    Wq = np.asarray(Wq, np.float32)
    Wk = np.asarray(Wk, np.float32)
    Wv = np.asarray(Wv, np.float32)
    bq = np.asarray(bq, np.float32)
    bk = np.asarray(bk, np.float32)
    bv = np.asarray(bv, np.float32)

    xts = [np.ascontiguousarray(hs[0].T), np.ascontiguousarray(hs[1].T)]
    mask_l = np.tril(np.ones((128, 128), np.float16))
    mask_u = np.triu(np.ones((128, 128), np.float16))
    masks = np.stack([mask_l, mask_u])

    in_maps = []
    for core in range(N_CORES):
        b = core // 4
        h0 = HPC * (core % 4)
        cq = slice(h0 * 64, (h0 + HPC) * 64)
        wqk = np.concatenate(
            [Wq[:, h0 * 64:(h0 + 2) * 64], Wk[:, h0 * 64:(h0 + 2) * 64],
             Wq[:, (h0 + 2) * 64:(h0 + 3) * 64], Wk[:, (h0 + 2) * 64:(h0 + 3) * 64]],
            axis=1)
        bqk = np.zeros((4, 128), np.float32)
        bqk[0] = bq[h0 * 64:(h0 + 2) * 64]
        bqk[1] = bk[h0 * 64:(h0 + 2) * 64]
        bqk[2, 0:64] = bq[(h0 + 2) * 64:(h0 + 3) * 64]
        bqk[3, 0:64] = bk[(h0 + 2) * 64:(h0 + 3) * 64]
        wv = np.zeros((E, 260), np.float32)
        wvr = np.zeros((1, 260), np.float32)
        for i in range(HPC):
            wv[:, 65 * i: 65 * i + 64] = Wv[:, (h0 + i) * 64:(h0 + i + 1) * 64]
            wvr[0, 65 * i: 65 * i + 64] = bv[(h0 + i) * 64:(h0 + i + 1) * 64]
            wvr[0, 65 * i + 64] = 1.0
        in_maps.append({
            "xt": xts[b],
            "wqk": np.ascontiguousarray(wqk),
            "bqk": np.ascontiguousarray(bqk),
            "wv": wv,
            "wvr": wvr,
            "masks": masks,
            "onesr": np.ones((1, 128), np.float32),
        })
    return in_maps


_NC_CACHE = None


def kernel(hidden_states, Wq, bq, Wk, bk, Wv, bv):
    global _NC_CACHE
    if _NC_CACHE is None:
        _NC_CACHE = build_program()
    nc = _NC_CACHE
    in_maps = make_in_maps(hidden_states, Wq, bq, Wk, bk, Wv, bv)
    res = bass_utils.run_bass_kernel_spmd(nc, in_maps, core_ids=list(range(N_CORES)))
    out = np.zeros((B, S, H * D), np.float32)
    for core in range(N_CORES):
        b = core // 4
        h0 = HPC * (core % 4)
        out[b, :, h0 * 64:(h0 + HPC) * 64] = res.results[core]["out"]
    return out



# revision 5
# speedup vs baseline: 1.8459x; 1.8459x over previous
"""LongFormer sliding-window attention on 8 Trainium2 NeuronCores.

Sharding: batch*heads data-parallel. 24 (batch, head) pairs -> 8 cores,
each core owns one batch (core//4) and 3 consecutive heads (3*(core%4)).
No collectives.

v2 design (cost-model driven):
  - Projections run as fp8e4 DoubleRow matmuls (0.5 cyc/row) over a
    CONCATENATED contraction [x8 ; x_residual8] x [W8 ; W_residual8]:
    error-compensated fp8 => product of (x8+xr8)(W8+Wr8), ~0.1% error,
    at 1/2 the streamed rows of bf16 and 1/4 of baseline fp32r.
  - Q/K written to SBUF as fp16 [dims, seq] (3 groups of 128 partitions:
    q01 | k01 | q2k2); head-2's K is realigned to partitions 0:63 by a
    small SBUF->SBUF DMA so scores matmuls keep matching base partitions.
  - Scores (transposed, [key, query]) per 256-query chunk go into ONE
    3-bank PSUM tile, tightly packed (1280 cols interior); one bracket
    (start..stop) per 2KB bank so the lazy PSUM zeroing stays correct.
  - ONE exp activation per chunk-head over the whole packed score strip
    (ACT cost is per free-column + fixed overhead, so merging exps is the
    main ACT win; ACT is the end-state bottleneck engine).
  - Triangle masks: two strided pair-multiplies (fp16, 2x DVE mode).
  - PV in fp16, 65-wide (64 dims + ones column for the denominator),
    the 3 heads accumulate into one PSUM bank per query half.
  - Normalization: batched reciprocal [128,3] + one broadcast
    tensor-multiply into the fp16 output tile.
  - Projection stripes are interleaved between attention chunks so PE
    stripe work fills the gaps while ACT (exp) is the bottleneck.
"""

import sys

import numpy as np
import ml_dtypes

sys.path.insert(0, "/opt/trn_rl_repo")

import concourse.bass as bass  # noqa: E402
import concourse.tile as tile  # noqa: E402
from concourse import bacc, mybir  # noqa: E402
from concourse import bass_utils  # noqa: E402

B, S, E = 2, 4096, 768
H, D = 12, 64
C = 16              # chunks of 256 queries
HPC = 3             # heads per core
N_CORES = 8
NT = 8              # 8 stripes of 512 tokens
KT2 = 12            # 6 kt blocks of x8 + 6 of xr8
VW = 195            # 3 heads x (64 dims + ones col)

f32 = mybir.dt.float32
f16 = mybir.dt.float16
f8 = mybir.dt.float8e4
DR = mybir.MatmulPerfMode.DoubleRow
FP8 = ml_dtypes.float8_e4m3fn


def _chunk_blocks(c):
    """Packed score layout for chunk c: list of (t, base_col, qlo, qn).

    t indexes the relative key tile (kt_abs = 2*(c-1)+t); base_col is the
    column of the block inside the packed PSUM strip; [qlo, qlo+qn) is the
    query span (relative to the chunk) the block covers.
    """
    if c == 0:
        return [(2, 0, 0, 256), (3, 256, 0, 256)]
    if c == C - 1:
        return [(0, 0, 0, 128), (1, 128, 0, 256), (2, 384, 0, 256),
                (3, 640, 0, 256)]
    return [(0, 0, 0, 128), (1, 128, 0, 256), (2, 384, 0, 256),
            (3, 640, 0, 256), (4, 896, 0, 256), (5, 1152, 128, 128)]


def _bank_pieces(base, qlo, qn):
    """Split [base, base+qn) on 512 (PSUM bank) boundaries.
    Returns (piece_col, piece_qlo, piece_n)."""
    out = []
    col, q, remaining = base, qlo, qn
    while remaining:
        n = min(remaining, 512 - col % 512)
        out.append((col, q, n))
        col += n
        q += n
        remaining -= n
    return out


def _build_body(tc, aps, has_vbias):
    nc = tc.nc
    xcat_d, wqk_d, wv_d, bqk_d, maskcat_d, wvr_d, out_d = aps

    from contextlib import ExitStack
    ctx = ExitStack()
    sb = ctx.enter_context(tc.tile_pool(name="sb", bufs=1))
    xp = ctx.enter_context(tc.tile_pool(name="xp", bufs=2))
    ep = ctx.enter_context(tc.tile_pool(name="ep", bufs=3))
    otp = ctx.enter_context(tc.tile_pool(name="otp", bufs=4))
    rcp = ctx.enter_context(tc.tile_pool(name="rcp", bufs=4))
    psA = ctx.enter_context(tc.tile_pool(name="psA", bufs=2, space="PSUM"))
    psB = ctx.enter_context(tc.tile_pool(name="psB", bufs=2, space="PSUM"))

    # ---- persistent SBUF tensors ----
    maskcat = sb.tile([128, 256], f16, tag="maskcat")
    nc.sync.dma_start(maskcat[:], maskcat_d[:])
    wqk = sb.tile([128, KT2, 384], f8, tag="wqk")
    nc.sync.dma_start(wqk[:], wqk_d[:])
    wv = sb.tile([128, KT2, 192], f8, tag="wv")
    nc.sync.dma_start(wv[:], wv_d[:])
    bqk = sb.tile([128, 3], f32, tag="bqk")
    nc.sync.dma_start(bqk[:], bqk_d[:])
    if has_vbias:
        wvr = sb.tile([1, 192], f16, tag="wvr")
        nc.sync.dma_start(wvr[:], wvr_d[:])
        ones1 = sb.tile([1, 128], f16, tag="ones1")
        nc.vector.memset(ones1[:], 1.0)

    qkT = sb.tile([128, 3 * S], f16, tag="qkT")    # q01 | k01 | q2k2
    k2sb = sb.tile([128, S], f16, tag="k2sb")      # rows 0:64 = head2 K
    vsb = sb.tile([128, 32 * VW], f16, tag="vsb")

    # static ones columns of vsb (denominator trick)
    ones_ap = vsb[:, 0:32 * VW].rearrange(
        "p (r h o) -> p r h o", r=32, h=HPC, o=65)[:, :, :, 64:65]
    nc.vector.memset(ones_ap, 1.0)

    # ---- projection stripes ----
    xcs = [None] * NT

    def prefetch(nt):
        xc = xp.tile([128, KT2, 512], f8, tag="xc", name="xc")
        nc.sync.dma_start(xc[:], xcat_d[:, :, nt * 512:(nt + 1) * 512])
        xcs[nt] = xc

    def do_stripe(nt):
        xc = xcs[nt]
        pq = psA.tile([128, 1536], f32, tag="psA", name="pq")
        # groups g0 = q01, g1 = k01, g2 = q2|k2 into banks 0/1/2 of pq
        for g in range(3):
            for j in range(6):
                nc.tensor.matmul(
                    pq[:, 512 * g:512 * g + 512],
                    wqk[:, 2 * j:2 * j + 2, 128 * g:128 * g + 128],
                    xc[:, 2 * j:2 * j + 2, :],
                    start=(j == 0), stop=(j == 5), perf_mode=DR,
                )
        # psum -> sbuf fp16 with bias add
        for g in range(3):
            nc.vector.tensor_scalar_add(
                qkT[:, g * S + nt * 512: g * S + nt * 512 + 512],
                pq[:, 512 * g:512 * g + 512], bqk[:, g:g + 1])
        # realign head-2 K rows 64:128 -> k2sb rows 0:64
        nc.sync.dma_start(
            k2sb[0:64, nt * 512:(nt + 1) * 512],
            qkT[64:128, 2 * S + nt * 512: 2 * S + nt * 512 + 512])
        # V projection for the stripe's 4 token tiles
        for rt4 in range(4):
            rt = nt * 4 + rt4
            pv = psB.tile([128, 512], f32, tag="psB", name="pv")
            for j in range(6):
                nc.tensor.matmul(
                    pv[:, 0:192],
                    xc[:, 2 * j:2 * j + 2, rt4 * 128:rt4 * 128 + 128],
                    wv[:, 2 * j:2 * j + 2, :],
                    start=(j == 0), stop=(False if has_vbias else j == 5),
                    perf_mode=DR,
                )
            if has_vbias:
                nc.tensor.matmul(pv[:, 0:192], ones1[:], wvr[:],
                                 start=False, stop=True)
            nc.vector.tensor_copy(
                vsb[:, rt * VW:(rt + 1) * VW].rearrange(
                    "p (h o) -> p h o", h=HPC, o=65)[:, :, 0:64],
                pv[:, 0:192].rearrange("p (h o) -> p h o", h=HPC, o=64))
        if nt + 2 < NT:
            prefetch(nt + 2)

    # ---- attention chunks ----
    def q_sl(hi, pos, n):
        if hi < 2:
            return qkT[64 * hi:64 * hi + 64, pos:pos + n]
        return qkT[0:64, 2 * S + pos: 2 * S + pos + n]

    def k_sl(hi, pos, n):
        if hi < 2:
            return qkT[64 * hi:64 * hi + 64, S + pos: S + pos + n]
        return k2sb[0:64, pos:pos + n]

    def do_chunk(c):
        blocks = _chunk_blocks(c)
        ncols = blocks[-1][1] + blocks[-1][3]
        pss = [None] * HPC
        ets = [None] * HPC

        def exp_mask(hi):
            et = ep.tile([128, 1408], f16, tag="et", name="et")
            nc.scalar.activation(et[:, 0:ncols], pss[hi][:, 0:ncols],
                                 mybir.ActivationFunctionType.Exp, scale=0.125)
            # triangle masks on diagonal blocks (pairs at stride 256)
            if c != 0:
                lo = et[:, 0:512].rearrange(
                    "p (a b) -> p a b", a=2, b=256)[:, :, 0:128]
                nc.vector.tensor_mul(
                    lo, lo,
                    maskcat[:, 0:128].unsqueeze(1).broadcast_to([128, 2, 128]))
            if c not in (0, C - 1):
                up = et[:, 896:1408].rearrange(
                    "p (a b) -> p a b", a=2, b=256)[:, :, 0:128]
                nc.vector.tensor_mul(
                    up, up,
                    maskcat[:, 128:256].unsqueeze(1).broadcast_to([128, 2, 128]))
            ets[hi] = et

        # scores for all heads, exp/mask trailing one head behind so the
        # PE never waits on ACT before it has queued independent work
        for hi in range(HPC):
            ps = psA.tile([128, 1536], f32, tag="psA", name="ps")
            pss[hi] = ps
            pieces = []
            for (t, base, qlo, qn) in blocks:
                kt_abs = 2 * (c - 1) + t
                for (col, q0, n) in _bank_pieces(base, qlo, qn):
                    pieces.append((col, n, kt_abs, q0))
            for i, (col, n, kt_abs, q0) in enumerate(pieces):
                bank = col // 512
                first = (i == 0) or (pieces[i - 1][0] // 512 != bank)
                last = (i == len(pieces) - 1) or \
                    (pieces[i + 1][0] // 512 != bank)
                nc.tensor.matmul(
                    ps[:, col:col + n],
                    k_sl(hi, kt_abs * 128, 128),
                    q_sl(hi, c * 256 + q0, n),
                    start=first, stop=last,
                )
            if hi >= 1:
                exp_mask(hi - 1)
        exp_mask(HPC - 1)

        # PV: 3 heads accumulate into one PSUM bank per query half
        pos = [psB.tile([128, 512], f32, tag="psB", name="po")
               for _ in range(2)]
        for hi in range(HPC):
            for qh in range(2):
                tlist = [(t, base, qlo) for (t, base, qlo, qn) in blocks
                         if (t <= 4 if qh == 0 else t >= 1)]
                for i, (t, base, qlo) in enumerate(tlist):
                    kt_abs = 2 * (c - 1) + t
                    col = base + 128 * qh - qlo
                    nc.tensor.matmul(
                        pos[qh][:, 65 * hi:65 * hi + 65],
                        ets[hi][:, col:col + 128],
                        vsb[:, kt_abs * VW + 65 * hi:
                            kt_abs * VW + 65 * hi + 65],
                        start=(hi == 0 and i == 0),
                        stop=(hi == HPC - 1 and i == len(tlist) - 1),
                    )
        # normalize + writeback
        for qh in range(2):
            po3 = pos[qh][:, 0:195].rearrange("p (h o) -> p h o", h=HPC, o=65)
            rec = rcp.tile([128, 3], f32, tag="rec", name="rec")
            nc.vector.reciprocal(rec[:, 0:3].unsqueeze(2), po3[:, :, 64:65])
            ot = otp.tile([128, 192], f16, tag="ot", name="ot")
            nc.vector.tensor_mul(
                ot[:, 0:192].rearrange("p (h o) -> p h o", h=HPC, o=64),
                po3[:, :, 0:64],
                rec[:, 0:3].unsqueeze(2).broadcast_to([128, 3, 64]))
            nc.sync.dma_start(
                out_d[c * 256 + qh * 128: c * 256 + qh * 128 + 128, :], ot[:])

    # ---- schedule: stripes interleaved with ready chunks ----
    prefetch(0)
    prefetch(1)
    for nt in range(NT):
        do_stripe(nt)
        if nt >= 1:
            do_chunk(2 * nt - 2)
            do_chunk(2 * nt - 1)
    do_chunk(14)
    do_chunk(15)
    ctx.close()


def build_program(has_vbias=False):
    nc = bacc.Bacc("TRN2", target_bir_lowering=False, debug=False)
    xcat_d = nc.dram_tensor("xcat", [128, KT2, S], f8, kind="ExternalInput").ap()
    wqk_d = nc.dram_tensor("wqk", [128, KT2, 384], f8, kind="ExternalInput").ap()
    wv_d = nc.dram_tensor("wv", [128, KT2, 192], f8, kind="ExternalInput").ap()
    bqk_d = nc.dram_tensor("bqk", [128, 3], f32, kind="ExternalInput").ap()
    maskcat_d = nc.dram_tensor("maskcat", [128, 256], f16,
                               kind="ExternalInput").ap()
    wvr_d = nc.dram_tensor("wvr", [1, 192], f16, kind="ExternalInput").ap() \
        if has_vbias else None
    out_d = nc.dram_tensor("out", [S, 192], f16, kind="ExternalOutput").ap()
    with tile.TileContext(nc) as tc:
        _build_body(tc, (xcat_d, wqk_d, wv_d, bqk_d, maskcat_d, wvr_d, out_d),
                    has_vbias)
    nc.compile()
    return nc


def _fp8_pair(a):
    a8 = a.astype(FP8)
    r8 = (a - a8.astype(np.float32)).astype(FP8)
    return a8, r8


def _stack_kt(a8, ar8, ncols):
    # [768, ncols] fp8 pair -> [128, 12, ncols]
    lo = np.ascontiguousarray(a8.reshape(6, 128, ncols).transpose(1, 0, 2))
    hi = np.ascontiguousarray(ar8.reshape(6, 128, ncols).transpose(1, 0, 2))
    return np.ascontiguousarray(np.concatenate([lo, hi], axis=1))


def make_in_maps(hidden_states, Wq, bq, Wk, bk, Wv, bv):
    hs = np.asarray(hidden_states, np.float32)
    Wq = np.asarray(Wq, np.float32)
    Wk = np.asarray(Wk, np.float32)
    Wv = np.asarray(Wv, np.float32)
    bq = np.asarray(bq, np.float32)
    bk = np.asarray(bk, np.float32)
    bv = np.asarray(bv, np.float32)

    xcats = []
    for b in range(B):
        x8, xr8 = _fp8_pair(np.ascontiguousarray(hs[b].T))
        xcats.append(_stack_kt(x8, xr8, S))

    maskcat = np.ascontiguousarray(np.concatenate(
        [np.tril(np.ones((128, 128), np.float16)),
         np.triu(np.ones((128, 128), np.float16))], axis=1))

    has_vbias = bool(np.any(bv != 0.0))

    in_maps = []
    for core in range(N_CORES):
        h0 = HPC * (core % 4)
        wcols = np.concatenate(
            [Wq[:, h0 * 64:(h0 + 2) * 64], Wk[:, h0 * 64:(h0 + 2) * 64],
             Wq[:, (h0 + 2) * 64:(h0 + 3) * 64],
             Wk[:, (h0 + 2) * 64:(h0 + 3) * 64]], axis=1)
        w8, wr8 = _fp8_pair(wcols)
        wqkcat = _stack_kt(w8, wr8, 384)
        v8, vr8 = _fp8_pair(np.ascontiguousarray(Wv[:, h0 * 64:(h0 + 3) * 64]))
        wvcat = _stack_kt(v8, vr8, 192)
        bqk = np.zeros((128, 3), np.float32)
        bqk[:, 0] = bq[h0 * 64:(h0 + 2) * 64]
        bqk[:, 1] = bk[h0 * 64:(h0 + 2) * 64]
        bqk[0:64, 2] = bq[(h0 + 2) * 64:(h0 + 3) * 64]
        bqk[64:128, 2] = bk[(h0 + 2) * 64:(h0 + 3) * 64]
        m = {
            "xcat": xcats[core // 4],
            "wqk": wqkcat,
            "wv": wvcat,
            "bqk": bqk,
            "maskcat": maskcat,
        }
        if has_vbias:
            m["wvr"] = np.ascontiguousarray(
                bv[h0 * 64:(h0 + 3) * 64].reshape(1, 192).astype(np.float16))
        in_maps.append(m)
    return in_maps


_NC_CACHE = None
_NC_CACHE_FLAG = None


def kernel(hidden_states, Wq, bq, Wk, bk, Wv, bv):
    global _NC_CACHE, _NC_CACHE_FLAG
    has_vbias = bool(np.any(np.asarray(bv) != 0.0))
    if _NC_CACHE is None or _NC_CACHE_FLAG != has_vbias:
        _NC_CACHE = build_program(has_vbias)
        _NC_CACHE_FLAG = has_vbias
    nc = _NC_CACHE
    in_maps = make_in_maps(hidden_states, Wq, bq, Wk, bk, Wv, bv)
    res = bass_utils.run_bass_kernel_spmd(nc, in_maps,
                                          core_ids=list(range(N_CORES)))
    out = np.zeros((B, S, H * D), np.float32)
    for core in range(N_CORES):
        b = core // 4
        h0 = HPC * (core % 4)
        out[b, :, h0 * 64:(h0 + HPC) * 64] = \
            res.results[core]["out"].astype(np.float32)
    return out
